# revision 9
# baseline (speedup 1.0000x reference)
"""GatedGraphNN Trainium2 kernel: 8-core SPMD, node-sharded with per-step AllGather.

v3 design notes (v1 5.17ms -> v2a 1.97ms -> this):
  - messages = h[col] @ W_msg.T + b_msg ; agg = segsum(messages, row). Linearity:
    agg @ W_ih.T = raw @ (W_ih W_msg).T + outer(deg, W_ih b_msg), raw = segsum(h[col]).
    So no per-edge matmul: gather h[col] (bf16), segment-sum via one-hot matmuls on PE,
    then fused dense GRU with W_c = W_ih @ W_msg.
  - Gather descriptor generation is Q7-bound (~8.4ns/idx per SWDGE queue):
    4 SWDGE queues (queue_num=res) -> 4 Q7 core-pairs generate concurrently.
  - Cells are (tile T, src-residue r, dest-quarter dq) padded to 128-slot chunks
    (single quarter per chunk: 1 matmul + 128 one-hot columns per chunk keeps
    DVE oh-gen and PE matmul count minimal); gather calls span the 4 dq cells
    of (T, r) on queue r.
  - AllGather is chunked 5x (5 dense tiles per chunk) with a chunk-major h_full
    layout [5, 8, 2560, H] so AG chunk k overlaps compute of later tiles; only
    the last chunk's AG is on the critical path between steps.
  - Step 0 gathers read from a replicated full-x input (no initial AllGather).
  - h kept bf16; epilogue mostly bf16 except the last step (fp32 out).
"""

import numpy as np
import ml_dtypes

BF16 = ml_dtypes.bfloat16
N, H, STEPS, NCORES = 100000, 128, 5, 8
NPAD = 102400
SHARD = NPAD // NCORES          # 12800
NT = SHARD // 512               # 25 dense tiles of 512 dests
NAG = 5                         # AllGather chunks per step
CROWS = SHARD // NAG            # 2560 rows per AG chunk
TPC = NT // NAG                 # 5 dense tiles per AG chunk


def _rowof(col):
    """h_full linear row for global node id (chunk-major AG layout)."""
    c = col // SHARD
    i = col % SHARD
    k = i // CROWS
    return k * (NCORES * CROWS) + c * CROWS + (i % CROWS)


def _preprocess(edge_index):
    row = np.asarray(edge_index[0]).astype(np.int64)
    col = np.asarray(edge_index[1]).astype(np.int64)
    core = row // SHARD
    rloc = row - core * SHARD
    T = rloc // 512
    offt = rloc % 512
    dq = offt // 128
    off7 = offt % 128
    hrow = _rowof(col)                      # source row in h_full layout
    res = hrow % 4
    gidx = hrow // 4                        # < 25600, fits int16
    cell = (T * 4 + res) * 4 + dq           # (T, res, dq), 400 cells
    NCELL = NT * 16

    cnt = np.zeros((NCORES, NCELL), np.int64)
    np.add.at(cnt, (core, cell), 1)
    nchunks = np.maximum(1, (cnt.max(axis=0) + 127) // 128)
    cellchunk0 = np.zeros(NCELL + 1, np.int64)
    cellchunk0[1:] = np.cumsum(nchunks)
    TOTC = int(cellchunk0[-1])

    deg = np.zeros((NCORES, SHARD), np.float32)
    np.add.at(deg, (core, rloc), 1.0)

    order = np.lexsort((col, cell, core))
    core_s, cell_s, off7_s, gidx_s = (
        core[order], cell[order], off7[order], gidx[order])

    idxflat = np.zeros((NCORES, TOTC * 128), np.int16)
    key = core_s * NCELL + cell_s
    bounds = np.flatnonzero(np.diff(key)) + 1
    starts = np.concatenate([[0], bounds])
    ends = np.concatenate([bounds, [len(key)]])
    pos = np.empty(len(key), np.int64)
    for st, en in zip(starts, ends):
        pos[st:en] = np.arange(en - st)
    slot_global = cellchunk0[cell_s] * 128 + pos
    idxflat[core_s, slot_global] = gidx_s.astype(np.int16)

    offs = np.full((NCORES, 128, TOTC), 999.0, np.float32)
    offs[core_s, slot_global % 128, slot_global // 128] = off7_s

    # per-tile mm structure: quarter-major so each PSUM region's accumulation
    # group is one contiguous run of matmuls.
    tiles = []
    for Ti in range(NT):
        entries = []
        for qq in range(4):
            for r in range(4):
                cell_i = (Ti * 4 + r) * 4 + qq
                call0 = int(cellchunk0[(Ti * 4 + r) * 4])
                for c in range(int(cellchunk0[cell_i]),
                               int(cellchunk0[cell_i + 1])):
                    entries.append([r, c - call0, qq, int(c), False, False])
        for qq in range(4):
            qe = [e for e in entries if e[2] == qq]
            qe[0][4] = True
            qe[-1][5] = True
        tiles.append(entries)

    # gather call sizes per (T, r): chunks of the 4 dq cells, contiguous
    callinfo = []
    for Ti in range(NT):
        for r in range(4):
            c0 = int(cellchunk0[(Ti * 4 + r) * 4])
            c1 = int(cellchunk0[(Ti * 4 + r) * 4 + 4])
            callinfo.append((c0, c1 - c0))
    maxch = max(n for _, n in callinfo)

    idx16 = np.zeros((NCORES, 128, TOTC * 8), np.int16)
    for c in range(NCORES):
        w = idxflat[c].reshape(TOTC * 8, 16).T
        idx16[c] = np.tile(w, (8, 1))

    return dict(idx16=idx16, offs=offs.astype(BF16), deg=deg,
                cellchunk0=cellchunk0, TOTC=TOTC, tiles=tiles,
                callinfo=callinfo, maxch=maxch)


def _build(pp):
    import concourse.bass as bass
    import concourse.bacc as bacc
    import concourse.mybir as mybir
    import concourse.tile as tile
    from concourse.bass import broadcast_tensor_aps

    TOTC, tiles, callinfo, maxch = (
        pp["TOTC"], pp["tiles"], pp["callinfo"], pp["maxch"])

    dt = mybir.dt
    AF = mybir.ActivationFunctionType
    OP = mybir.AluOpType
    nc = bacc.Bacc(num_devices=NCORES, num_swdge_queues=4,
                   dynamic_dma_scratch_size=32768)
    RG = [list(range(NCORES))]

    x_T = nc.dram_tensor("x_T", [H, SHARD], dt.bfloat16, kind="ExternalInput")
    xfull_d = nc.dram_tensor("xfull", [NPAD, H], dt.bfloat16, kind="ExternalInput")
    idx_d = nc.dram_tensor("idx", [128, TOTC * 8], dt.int16, kind="ExternalInput")
    offs_d = nc.dram_tensor("offs", [128, TOTC], dt.bfloat16, kind="ExternalInput")
    deg_d = nc.dram_tensor("deg", [1, SHARD], dt.bfloat16, kind="ExternalInput")
    wct_d = nc.dram_tensor("wct", [H, 3 * H], dt.bfloat16, kind="ExternalInput")
    whht_d = nc.dram_tensor("whht", [H, 3 * H], dt.bfloat16, kind="ExternalInput")
    v3_d = nc.dram_tensor("v3", [1, 3 * H], dt.bfloat16, kind="ExternalInput")
    bias_d = nc.dram_tensor("bias", [H, 4], dt.float32, kind="ExternalInput")
    iota_d = nc.dram_tensor("iota", [H, H], dt.bfloat16, kind="ExternalInput")
    idn_d = nc.dram_tensor("idn", [H, H], dt.bfloat16, kind="ExternalInput")
    idnf_d = nc.dram_tensor("idnf", [H, H], dt.float32, kind="ExternalInput")
    out_d = nc.dram_tensor("out", [SHARD, H], dt.float32, kind="ExternalOutput")

    # double-buffered: step s gathers from h_fulls[(s-1) % 2] (or xfull at s=0)
    # while its chunked AllGathers write h_fulls[s % 2] -- chunk k's AG starts
    # as soon as tiles 5k..5k+4 have produced their new h rows.
    h_fulls = [
        nc.dram_tensor(f"h_full{i}", [NPAD, H], dt.bfloat16, kind="Internal",
                       addr_space="Shared")
        for i in range(2)
    ]
    bounce = nc.dram_tensor("bounce", [SHARD, H], dt.bfloat16, kind="Internal")

    with tile.TileContext(nc) as tc:
        with (
            tc.tile_pool(name="res", bufs=1) as res,
            tc.tile_pool(name="gath", bufs=3) as gpool,
            tc.tile_pool(name="oh", bufs=2) as ohpool,
            tc.tile_pool(name="agg", bufs=2) as apool,
            tc.tile_pool(name="epi", bufs=2) as epool,
            tc.tile_pool(name="stg", bufs=2) as spool,
            tc.tile_pool(name="pseg", bufs=2, space="PSUM") as pseg,
            tc.tile_pool(name="pden", bufs=1, space="PSUM") as pden,
            tc.tile_pool(name="ptr", bufs=1, space="PSUM") as ptr,
        ):
            def ld(dram, shape, dtype, name):
                t = res.tile(shape, dtype, tag=name)
                nc.sync.dma_start(t[:], dram[:, :])
                return t

            idx_sb = ld(idx_d, [128, TOTC * 8], dt.int16, "idx")
            offs_sb = ld(offs_d, [128, TOTC], dt.bfloat16, "offs")
            deg_sb = ld(deg_d, [1, SHARD], dt.bfloat16, "deg")
            wct = ld(wct_d, [H, 3 * H], dt.bfloat16, "wct")
            whht = ld(whht_d, [H, 3 * H], dt.bfloat16, "whht")
            v3 = ld(v3_d, [1, 3 * H], dt.bfloat16, "v3")
            bias = ld(bias_d, [H, 4], dt.float32, "bias")
            iota = ld(iota_d, [H, H], dt.bfloat16, "iota")
            idn = ld(idn_d, [H, H], dt.bfloat16, "idn")
            idnf = ld(idnf_d, [H, H], dt.float32, "idnf")

            hT = [res.tile([H, SHARD], dt.bfloat16, tag=f"hT{i}", name=f"hT{i}")
                  for i in range(2)]
            nc.sync.dma_start(hT[0][:], x_T[:, :])

            b_r = bias[:, 0:1]
            b_z = bias[:, 1:2]
            b_in = bias[:, 2:3]
            b_hn = bias[:, 3:4]

            for s in range(STEPS):
                hcur, hnxt = hT[s % 2], hT[(s + 1) % 2]
                last = s == STEPS - 1
                src_d = xfull_d if s == 0 else h_fulls[(s - 1) % 2]
                ag_dst = h_fulls[s % 2]
                for T in range(NT):
                    gt = {}
                    oht = {}
                    call0 = {}
                    for r in range(4):
                        c0, nch = callinfo[T * 4 + r]
                        g = gpool.tile([128, maxch, H], dt.bfloat16, tag=f"g{r}")
                        n_idx = nch * 128
                        nc.gpsimd.dma_gather(
                            g[:, 0:nch, :],
                            src_d[r::4, :],
                            idx_sb[:, c0 * 8:(c0 + nch) * 8],
                            n_idx, n_idx, H, elem_step=4 * H, queue_num=r)
                        gt[r] = g
                        oh = ohpool.tile([128, nch * 128], dt.bfloat16,
                                         tag=f"oh{r}")
                        a_in, b_in2 = broadcast_tensor_aps(
                            offs_sb[:, c0:c0 + nch, None], iota[:, None, :])
                        nc.vector.tensor_tensor(
                            oh[:].rearrange("p (m f) -> p m f", m=nch),
                            a_in, b_in2, OP.is_equal)
                        oht[r] = oh
                        call0[r] = c0

                    ps = pseg.tile([H, 512], dt.float32, tag="pseg")
                    for r, c, qq, gc, st, sp in tiles[T]:
                        nc.tensor.matmul(
                            ps[:, qq * 128:(qq + 1) * 128],
                            gt[r][:, c, :],
                            oht[r][:, c * 128:(c + 1) * 128],
                            start=st, stop=sp, skip_group_check=True)

                    ragg = apool.tile([H, 512], dt.bfloat16, tag="ragg")
                    nc.scalar.copy(ragg[:], ps[:])

                    hsl = hcur[:, T * 512:(T + 1) * 512]
                    dsl = deg_sb[0:1, T * 512:(T + 1) * 512]
                    p_r = pden.tile([H, 512], dt.float32, tag="p_r")
                    p_z = pden.tile([H, 512], dt.float32, tag="p_z")
                    p_in = pden.tile([H, 512], dt.float32, tag="p_in")
                    p_hn = pden.tile([H, 512], dt.float32, tag="p_hn")
                    nc.tensor.matmul(p_r[:], wct[:, 0:128], ragg[:], start=True, stop=False)
                    nc.tensor.matmul(p_r[:], whht[:, 0:128], hsl, start=False, stop=False)
                    nc.tensor.matmul(p_r[:], v3[0:1, 0:128], dsl, start=False, stop=True)
                    nc.tensor.matmul(p_z[:], wct[:, 128:256], ragg[:], start=True, stop=False)
                    nc.tensor.matmul(p_z[:], whht[:, 128:256], hsl, start=False, stop=False)
                    nc.tensor.matmul(p_z[:], v3[0:1, 128:256], dsl, start=False, stop=True)
                    nc.tensor.matmul(p_in[:], wct[:, 256:384], ragg[:], start=True, stop=False)
                    nc.tensor.matmul(p_in[:], v3[0:1, 256:384], dsl, start=False, stop=True)
                    nc.tensor.matmul(p_hn[:], whht[:, 256:384], hsl, start=True, stop=True)

                    if not last:
                        r_t = epool.tile([H, 512], dt.bfloat16, tag="r")
                        z_t = epool.tile([H, 512], dt.bfloat16, tag="z")
                        ghn = epool.tile([H, 512], dt.bfloat16, tag="ghn")
                        pin = epool.tile([H, 512], dt.bfloat16, tag="pin")
                        t2 = epool.tile([H, 512], dt.bfloat16, tag="t2")
                        pre_n = epool.tile([H, 512], dt.bfloat16, tag="pre_n")
                        nn = epool.tile([H, 512], dt.bfloat16, tag="nn")
                        am = epool.tile([H, 512], dt.bfloat16, tag="am")
                        bm = epool.tile([H, 512], dt.bfloat16, tag="bm")

                        nc.scalar.activation(r_t[:], p_r[:], AF.Sigmoid, bias=b_r)
                        nc.scalar.activation(z_t[:], p_z[:], AF.Sigmoid, bias=b_z)
                        nc.scalar.activation(ghn[:], p_hn[:], AF.Identity, bias=b_hn)
                        nc.scalar.activation(pin[:], p_in[:], AF.Identity, bias=b_in)
                        nc.vector.tensor_tensor(t2[:], r_t[:], ghn[:], OP.mult)
                        nc.vector.tensor_tensor(pre_n[:], t2[:], pin[:], OP.add)
                        nc.scalar.activation(nn[:], pre_n[:], AF.Tanh)
                        nc.vector.tensor_tensor(am[:], hsl, nn[:], OP.subtract)
                        nc.vector.tensor_tensor(bm[:], z_t[:], am[:], OP.mult)
                        hn_sl = hnxt[:, T * 512:(T + 1) * 512]
                        nc.vector.tensor_tensor(hn_sl, bm[:], nn[:], OP.add)
                        stg = spool.tile([128, 4, H], dt.bfloat16, tag="stg")
                        for j in range(4):
                            pt = ptr.tile([128, 128], dt.bfloat16, tag="pt")
                            nc.tensor.transpose(
                                pt[:], hnxt[:, T * 512 + j * 128: T * 512 + (j + 1) * 128],
                                idn[:])
                            nc.scalar.copy(stg[:, j, :], pt[:])
                        nc.sync.dma_start(
                            bounce.rearrange("(t g p) f -> t p g f", p=128, g=4)[T],
                            stg[:])
                    else:
                        r = epool.tile([H, 512], dt.float32, tag="rf")
                        z = epool.tile([H, 512], dt.float32, tag="zf")
                        ghn = epool.tile([H, 512], dt.float32, tag="ghnf")
                        t2 = epool.tile([H, 512], dt.float32, tag="t2f", bufs=1)
                        pre_n = epool.tile([H, 512], dt.float32, tag="pre_nf", bufs=1)
                        nn = epool.tile([H, 512], dt.float32, tag="nnf")
                        am = epool.tile([H, 512], dt.float32, tag="amf", bufs=1)
                        bm = epool.tile([H, 512], dt.float32, tag="bmf", bufs=1)

                        nc.scalar.activation(r[:], p_r[:], AF.Sigmoid, bias=b_r)
                        nc.scalar.activation(z[:], p_z[:], AF.Sigmoid, bias=b_z)
                        nc.scalar.activation(ghn[:], p_hn[:], AF.Identity, bias=b_hn)
                        nc.vector.scalar_tensor_tensor(
                            t2[:], r[:], 0.0, ghn[:], OP.add, OP.mult)
                        nc.vector.tensor_tensor(pre_n[:], t2[:], p_in[:], OP.add)
                        nc.scalar.activation(nn[:], pre_n[:], AF.Tanh, bias=b_in)
                        nc.vector.tensor_tensor(am[:], hsl, nn[:], OP.subtract)
                        nc.vector.scalar_tensor_tensor(
                            bm[:], z[:], 0.0, am[:], OP.add, OP.mult)
                        hf = epool.tile([H, 512], dt.float32, tag="hf", bufs=2)
                        nc.vector.tensor_tensor(hf[:], bm[:], nn[:], OP.add)
                        stgf = spool.tile([128, 4, H], dt.float32, tag="stgf")
                        for j in range(4):
                            ptf = ptr.tile([128, 128], dt.float32, tag="ptf")
                            nc.tensor.matmul(ptf[:], hf[:, j * 128:(j + 1) * 128],
                                             idnf[:], is_transpose=True)
                            nc.scalar.copy(stgf[:, j, :], ptf[:])
                        nc.sync.dma_start(
                            out_d.rearrange("(t g p) f -> t p g f", p=128, g=4)[T],
                            stgf[:])

                    # AG chunk k is emitted 2 tiles after its data is ready so
                    # the POOL sequencer's wait on the bounce writes is already
                    # satisfied and gather dispatch doesn't stall.
                    if not last:
                        if TPC - 1 <= T - 2 and (T - 2) % TPC == TPC - 1:
                            k = (T - 2) // TPC
                        elif T == NT - 1:
                            k = NAG - 1
                        else:
                            k = None
                        if k is not None:
                            nc.gpsimd.collective_compute(
                                "AllGather", OP.bypass, replica_groups=RG,
                                ins=[bounce[k * CROWS:(k + 1) * CROWS, :]],
                                outs=[ag_dst[k * NCORES * CROWS:(k + 1) * NCORES * CROWS, :]])
    nc.finalize()
    return nc


_CACHE = {}
_last_in_maps = None


def kernel(**inputs):
    x = np.asarray(inputs["x"], np.float32)
    edge_index = np.asarray(inputs["edge_index"])
    W_msg = np.asarray(inputs["W_msg"], np.float32)
    b_msg = np.asarray(inputs["b_msg"], np.float32)
    W_ih = np.asarray(inputs["W_ih"], np.float32)
    W_hh = np.asarray(inputs["W_hh"], np.float32)
    b_ih = np.asarray(inputs["b_ih"], np.float32)
    b_hh = np.asarray(inputs["b_hh"], np.float32)

    pp = _preprocess(edge_index)
    key = (pp["TOTC"], tuple(c for c, _ in pp["callinfo"]))
    if key not in _CACHE:
        _CACHE[key] = _build(pp)
    nc = _CACHE[key]

    xp = np.zeros((NPAD, H), np.float32)
    xp[:N] = x
    # xfull in chunk-major h_full layout
    perm = _rowof(np.arange(NPAD))
    xfull = np.empty((NPAD, H), np.float32)
    xfull[perm] = xp
    xfull = xfull.astype(BF16)
    W_c = W_ih @ W_msg
    v3 = (W_ih @ b_msg).reshape(1, 3 * H)
    bias = np.stack([
        b_ih[0:128] + b_hh[0:128],
        b_ih[128:256] + b_hh[128:256],
        b_ih[256:384],
        b_hh[256:384],
    ], axis=1).astype(np.float32)
    iota = np.broadcast_to(np.arange(H, dtype=np.float32), (H, H)).astype(BF16)
    idn = np.eye(H, dtype=np.float32)

    in_maps = []
    for c in range(NCORES):
        sh = xp[c * SHARD:(c + 1) * SHARD]
        in_maps.append({
            "x_T": np.ascontiguousarray(sh.T).astype(BF16),
            "xfull": xfull,
            "idx": pp["idx16"][c],
            "offs": pp["offs"][c],
            "deg": pp["deg"][c].reshape(1, SHARD).astype(BF16),
            "wct": np.ascontiguousarray(W_c.T).astype(BF16),
            "whht": np.ascontiguousarray(W_hh.T).astype(BF16),
            "v3": v3.astype(BF16),
            "bias": bias,
            "iota": np.ascontiguousarray(iota),
            "idn": idn.astype(BF16),
            "idnf": idn,
        })

    global _last_in_maps
    _last_in_maps = in_maps
    from concourse.bass_utils import run_bass_kernel_spmd
    res = run_bass_kernel_spmd(nc, in_maps, core_ids=list(range(NCORES)))
    outs = res.results
    full = np.concatenate([outs[c]["out"] for c in range(NCORES)], axis=0)
    return full[:N].astype(np.float32)


# revision 10
# speedup vs baseline: 1.0684x; 1.0684x over previous
"""GatedGraphNN Trainium2 kernel: 8-core SPMD, node-sharded with per-step AllGather.

v3 design notes (v1 5.17ms -> v2a 1.97ms -> this):
  - messages = h[col] @ W_msg.T + b_msg ; agg = segsum(messages, row). Linearity:
    agg @ W_ih.T = raw @ (W_ih W_msg).T + outer(deg, W_ih b_msg), raw = segsum(h[col]).
    So no per-edge matmul: gather h[col] (bf16), segment-sum via one-hot matmuls on PE,
    then fused dense GRU with W_c = W_ih @ W_msg.
  - Gather descriptor generation is Q7-bound (~8.4ns/idx per SWDGE queue):
    4 SWDGE queues (queue_num=res) -> 4 Q7 core-pairs generate concurrently.
  - Cells are (tile T, src-residue r, dest-quarter dq) padded to 128-slot chunks
    (single quarter per chunk: 1 matmul + 128 one-hot columns per chunk keeps
    DVE oh-gen and PE matmul count minimal); gather calls span the 4 dq cells
    of (T, r) on queue r.
  - AllGather is chunked 5x (5 dense tiles per chunk) with a chunk-major h_full
    layout [5, 8, 2560, H] so AG chunk k overlaps compute of later tiles; only
    the last chunk's AG is on the critical path between steps.
  - Step 0 gathers read from a replicated full-x input (no initial AllGather).
  - h kept bf16; epilogue mostly bf16 except the last step (fp32 out).
"""

import numpy as np
import ml_dtypes

BF16 = ml_dtypes.bfloat16
N, H, STEPS, NCORES = 100000, 128, 5, 8
NPAD = 102400
SHARD = NPAD // NCORES          # 12800
NT = SHARD // 512               # 25 dense tiles of 512 dests
NAG = 1                         # AllGather chunks per step (chunked AG loses:
                                # concurrent AGs run ~3-4x slower from SDMA/HBM
                                # contention with the gather storm)
CROWS = SHARD // NAG            # 2560 rows per AG chunk
TPC = NT // NAG                 # 5 dense tiles per AG chunk


def _rowof(col):
    """h_full linear row for global node id (chunk-major AG layout)."""
    c = col // SHARD
    i = col % SHARD
    k = i // CROWS
    return k * (NCORES * CROWS) + c * CROWS + (i % CROWS)


def _preprocess(edge_index):
    row = np.asarray(edge_index[0]).astype(np.int64)
    col = np.asarray(edge_index[1]).astype(np.int64)
    core = row // SHARD
    rloc = row - core * SHARD
    T = rloc // 512
    offt = rloc % 512
    dq = offt // 128
    off7 = offt % 128
    hrow = _rowof(col)                      # source row in h_full layout
    res = hrow % 4
    gidx = hrow // 4                        # < 25600, fits int16
    cell = (T * 4 + res) * 4 + dq           # (T, res, dq), 400 cells
    NCELL = NT * 16

    cnt = np.zeros((NCORES, NCELL), np.int64)
    np.add.at(cnt, (core, cell), 1)
    nchunks = np.maximum(1, (cnt.max(axis=0) + 127) // 128)
    cellchunk0 = np.zeros(NCELL + 1, np.int64)
    cellchunk0[1:] = np.cumsum(nchunks)
    TOTC = int(cellchunk0[-1])

    deg = np.zeros((NCORES, SHARD), np.float32)
    np.add.at(deg, (core, rloc), 1.0)

    order = np.lexsort((col, cell, core))
    core_s, cell_s, off7_s, gidx_s = (
        core[order], cell[order], off7[order], gidx[order])

    idxflat = np.zeros((NCORES, TOTC * 128), np.int16)
    key = core_s * NCELL + cell_s
    bounds = np.flatnonzero(np.diff(key)) + 1
    starts = np.concatenate([[0], bounds])
    ends = np.concatenate([bounds, [len(key)]])
    pos = np.empty(len(key), np.int64)
    for st, en in zip(starts, ends):
        pos[st:en] = np.arange(en - st)
    slot_global = cellchunk0[cell_s] * 128 + pos
    idxflat[core_s, slot_global] = gidx_s.astype(np.int16)

    offs = np.full((NCORES, 128, TOTC), 999.0, np.float32)
    offs[core_s, slot_global % 128, slot_global // 128] = off7_s

    # per-tile mm structure: quarter-major so each PSUM region's accumulation
    # group is one contiguous run of matmuls.
    tiles = []
    for Ti in range(NT):
        entries = []
        for qq in range(4):
            for r in range(4):
                cell_i = (Ti * 4 + r) * 4 + qq
                call0 = int(cellchunk0[(Ti * 4 + r) * 4])
                for c in range(int(cellchunk0[cell_i]),
                               int(cellchunk0[cell_i + 1])):
                    entries.append([r, c - call0, qq, int(c), False, False])
        for qq in range(4):
            qe = [e for e in entries if e[2] == qq]
            qe[0][4] = True
            qe[-1][5] = True
        tiles.append(entries)

    # gather call sizes per (T, r): chunks of the 4 dq cells, contiguous
    callinfo = []
    for Ti in range(NT):
        for r in range(4):
            c0 = int(cellchunk0[(Ti * 4 + r) * 4])
            c1 = int(cellchunk0[(Ti * 4 + r) * 4 + 4])
            callinfo.append((c0, c1 - c0))
    maxch = max(n for _, n in callinfo)

    idx16 = np.zeros((NCORES, 128, TOTC * 8), np.int16)
    for c in range(NCORES):
        w = idxflat[c].reshape(TOTC * 8, 16).T
        idx16[c] = np.tile(w, (8, 1))

    return dict(idx16=idx16, offs=offs.astype(BF16), deg=deg,
                cellchunk0=cellchunk0, TOTC=TOTC, tiles=tiles,
                callinfo=callinfo, maxch=maxch)


def _build(pp):
    import concourse.bass as bass
    import concourse.bacc as bacc
    import concourse.mybir as mybir
    import concourse.tile as tile
    from concourse.bass import broadcast_tensor_aps

    TOTC, tiles, callinfo, maxch = (
        pp["TOTC"], pp["tiles"], pp["callinfo"], pp["maxch"])

    dt = mybir.dt
    AF = mybir.ActivationFunctionType
    OP = mybir.AluOpType
    nc = bacc.Bacc(num_devices=NCORES, num_swdge_queues=4,
                   dynamic_dma_scratch_size=32768)
    RG = [list(range(NCORES))]

    x_T = nc.dram_tensor("x_T", [H, SHARD], dt.bfloat16, kind="ExternalInput")
    xfull_d = nc.dram_tensor("xfull", [NPAD, H], dt.bfloat16, kind="ExternalInput")
    idx_d = nc.dram_tensor("idx", [128, TOTC * 8], dt.int16, kind="ExternalInput")
    offs_d = nc.dram_tensor("offs", [128, TOTC], dt.bfloat16, kind="ExternalInput")
    deg_d = nc.dram_tensor("deg", [1, SHARD], dt.bfloat16, kind="ExternalInput")
    wct_d = nc.dram_tensor("wct", [H, 3 * H], dt.bfloat16, kind="ExternalInput")
    whht_d = nc.dram_tensor("whht", [H, 3 * H], dt.bfloat16, kind="ExternalInput")
    v3_d = nc.dram_tensor("v3", [1, 3 * H], dt.bfloat16, kind="ExternalInput")
    bias_d = nc.dram_tensor("bias", [H, 4], dt.float32, kind="ExternalInput")
    iota_d = nc.dram_tensor("iota", [H, H], dt.bfloat16, kind="ExternalInput")
    idn_d = nc.dram_tensor("idn", [H, H], dt.bfloat16, kind="ExternalInput")
    idnf_d = nc.dram_tensor("idnf", [H, H], dt.float32, kind="ExternalInput")
    out_d = nc.dram_tensor("out", [SHARD, H], dt.float32, kind="ExternalOutput")

    # double-buffered: step s gathers from h_fulls[(s-1) % 2] (or xfull at s=0)
    # while its chunked AllGathers write h_fulls[s % 2] -- chunk k's AG starts
    # as soon as tiles 5k..5k+4 have produced their new h rows.
    h_fulls = [
        nc.dram_tensor(f"h_full{i}", [NPAD, H], dt.bfloat16, kind="Internal",
                       addr_space="Shared")
        for i in range(2)
    ]
    bounce = nc.dram_tensor("bounce", [SHARD, H], dt.bfloat16, kind="Internal")

    with tile.TileContext(nc) as tc:
        with (
            tc.tile_pool(name="res", bufs=1) as res,
            tc.tile_pool(name="gath", bufs=3) as gpool,
            tc.tile_pool(name="oh", bufs=2) as ohpool,
            tc.tile_pool(name="agg", bufs=2) as apool,
            tc.tile_pool(name="epi", bufs=2) as epool,
            tc.tile_pool(name="stg", bufs=2) as spool,
            tc.tile_pool(name="pseg", bufs=2, space="PSUM") as pseg,
            tc.tile_pool(name="pden", bufs=1, space="PSUM") as pden,
            tc.tile_pool(name="ptr", bufs=1, space="PSUM") as ptr,
        ):
            def ld(dram, shape, dtype, name):
                t = res.tile(shape, dtype, tag=name)
                nc.sync.dma_start(t[:], dram[:, :])
                return t

            idx_sb = ld(idx_d, [128, TOTC * 8], dt.int16, "idx")
            offs_sb = ld(offs_d, [128, TOTC], dt.bfloat16, "offs")
            deg_sb = ld(deg_d, [1, SHARD], dt.bfloat16, "deg")
            wct = ld(wct_d, [H, 3 * H], dt.bfloat16, "wct")
            whht = ld(whht_d, [H, 3 * H], dt.bfloat16, "whht")
            v3 = ld(v3_d, [1, 3 * H], dt.bfloat16, "v3")
            bias = ld(bias_d, [H, 4], dt.float32, "bias")
            iota = ld(iota_d, [H, H], dt.bfloat16, "iota")
            idn = ld(idn_d, [H, H], dt.bfloat16, "idn")
            idnf = ld(idnf_d, [H, H], dt.float32, "idnf")

            hT = [res.tile([H, SHARD], dt.bfloat16, tag=f"hT{i}", name=f"hT{i}")
                  for i in range(2)]
            nc.sync.dma_start(hT[0][:], x_T[:, :])

            b_r = bias[:, 0:1]
            b_z = bias[:, 1:2]
            b_in = bias[:, 2:3]
            b_hn = bias[:, 3:4]

            for s in range(STEPS):
                hcur, hnxt = hT[s % 2], hT[(s + 1) % 2]
                last = s == STEPS - 1
                src_d = xfull_d if s == 0 else h_fulls[(s - 1) % 2]
                ag_dst = h_fulls[s % 2]
                for T in range(NT):
                    gt = {}
                    oht = {}
                    call0 = {}
                    for r in range(4):
                        c0, nch = callinfo[T * 4 + r]
                        g = gpool.tile([128, maxch, H], dt.bfloat16, tag=f"g{r}")
                        n_idx = nch * 128
                        nc.gpsimd.dma_gather(
                            g[:, 0:nch, :],
                            src_d[r::4, :],
                            idx_sb[:, c0 * 8:(c0 + nch) * 8],
                            n_idx, n_idx, H, elem_step=4 * H, queue_num=r)
                        gt[r] = g
                        oh = ohpool.tile([128, nch * 128], dt.bfloat16,
                                         tag=f"oh{r}")
                        a_in, b_in2 = broadcast_tensor_aps(
                            offs_sb[:, c0:c0 + nch, None], iota[:, None, :])
                        nc.vector.tensor_tensor(
                            oh[:].rearrange("p (m f) -> p m f", m=nch),
                            a_in, b_in2, OP.is_equal)
                        oht[r] = oh
                        call0[r] = c0

                    ps = pseg.tile([H, 512], dt.float32, tag="pseg")
                    for r, c, qq, gc, st, sp in tiles[T]:
                        nc.tensor.matmul(
                            ps[:, qq * 128:(qq + 1) * 128],
                            gt[r][:, c, :],
                            oht[r][:, c * 128:(c + 1) * 128],
                            start=st, stop=sp, skip_group_check=True)

                    ragg = apool.tile([H, 512], dt.bfloat16, tag="ragg")
                    nc.scalar.copy(ragg[:], ps[:])

                    hsl = hcur[:, T * 512:(T + 1) * 512]
                    dsl = deg_sb[0:1, T * 512:(T + 1) * 512]
                    p_r = pden.tile([H, 512], dt.float32, tag="p_r")
                    p_z = pden.tile([H, 512], dt.float32, tag="p_z")
                    p_in = pden.tile([H, 512], dt.float32, tag="p_in")
                    p_hn = pden.tile([H, 512], dt.float32, tag="p_hn")
                    nc.tensor.matmul(p_r[:], wct[:, 0:128], ragg[:], start=True, stop=False)
                    nc.tensor.matmul(p_r[:], whht[:, 0:128], hsl, start=False, stop=False)
                    nc.tensor.matmul(p_r[:], v3[0:1, 0:128], dsl, start=False, stop=True)
                    nc.tensor.matmul(p_z[:], wct[:, 128:256], ragg[:], start=True, stop=False)
                    nc.tensor.matmul(p_z[:], whht[:, 128:256], hsl, start=False, stop=False)
                    nc.tensor.matmul(p_z[:], v3[0:1, 128:256], dsl, start=False, stop=True)
                    nc.tensor.matmul(p_in[:], wct[:, 256:384], ragg[:], start=True, stop=False)
                    nc.tensor.matmul(p_in[:], v3[0:1, 256:384], dsl, start=False, stop=True)
                    nc.tensor.matmul(p_hn[:], whht[:, 256:384], hsl, start=True, stop=True)

                    if not last:
                        r_t = epool.tile([H, 512], dt.bfloat16, tag="r")
                        z_t = epool.tile([H, 512], dt.bfloat16, tag="z")
                        ghn = epool.tile([H, 512], dt.bfloat16, tag="ghn")
                        pin = epool.tile([H, 512], dt.bfloat16, tag="pin")
                        t2 = epool.tile([H, 512], dt.bfloat16, tag="t2")
                        pre_n = epool.tile([H, 512], dt.bfloat16, tag="pre_n")
                        nn = epool.tile([H, 512], dt.bfloat16, tag="nn")
                        am = epool.tile([H, 512], dt.bfloat16, tag="am")
                        bm = epool.tile([H, 512], dt.bfloat16, tag="bm")

                        nc.scalar.activation(r_t[:], p_r[:], AF.Sigmoid, bias=b_r)
                        nc.scalar.activation(z_t[:], p_z[:], AF.Sigmoid, bias=b_z)
                        nc.scalar.activation(ghn[:], p_hn[:], AF.Identity, bias=b_hn)
                        nc.scalar.activation(pin[:], p_in[:], AF.Identity, bias=b_in)
                        nc.vector.tensor_tensor(t2[:], r_t[:], ghn[:], OP.mult)
                        nc.vector.tensor_tensor(pre_n[:], t2[:], pin[:], OP.add)
                        nc.scalar.activation(nn[:], pre_n[:], AF.Tanh)
                        nc.vector.tensor_tensor(am[:], hsl, nn[:], OP.subtract)
                        nc.vector.tensor_tensor(bm[:], z_t[:], am[:], OP.mult)
                        hn_sl = hnxt[:, T * 512:(T + 1) * 512]
                        nc.vector.tensor_tensor(hn_sl, bm[:], nn[:], OP.add)
                        stg = spool.tile([128, 4, H], dt.bfloat16, tag="stg")
                        for j in range(4):
                            pt = ptr.tile([128, 128], dt.bfloat16, tag="pt")
                            nc.tensor.transpose(
                                pt[:], hnxt[:, T * 512 + j * 128: T * 512 + (j + 1) * 128],
                                idn[:])
                            nc.scalar.copy(stg[:, j, :], pt[:])
                        nc.sync.dma_start(
                            bounce.rearrange("(t g p) f -> t p g f", p=128, g=4)[T],
                            stg[:])
                    else:
                        r = epool.tile([H, 512], dt.float32, tag="rf")
                        z = epool.tile([H, 512], dt.float32, tag="zf")
                        ghn = epool.tile([H, 512], dt.float32, tag="ghnf")
                        t2 = epool.tile([H, 512], dt.float32, tag="t2f", bufs=1)
                        pre_n = epool.tile([H, 512], dt.float32, tag="pre_nf", bufs=1)
                        nn = epool.tile([H, 512], dt.float32, tag="nnf")
                        am = epool.tile([H, 512], dt.float32, tag="amf", bufs=1)
                        bm = epool.tile([H, 512], dt.float32, tag="bmf", bufs=1)

                        nc.scalar.activation(r[:], p_r[:], AF.Sigmoid, bias=b_r)
                        nc.scalar.activation(z[:], p_z[:], AF.Sigmoid, bias=b_z)
                        nc.scalar.activation(ghn[:], p_hn[:], AF.Identity, bias=b_hn)
                        nc.vector.scalar_tensor_tensor(
                            t2[:], r[:], 0.0, ghn[:], OP.add, OP.mult)
                        nc.vector.tensor_tensor(pre_n[:], t2[:], p_in[:], OP.add)
                        nc.scalar.activation(nn[:], pre_n[:], AF.Tanh, bias=b_in)
                        nc.vector.tensor_tensor(am[:], hsl, nn[:], OP.subtract)
                        nc.vector.scalar_tensor_tensor(
                            bm[:], z[:], 0.0, am[:], OP.add, OP.mult)
                        hf = epool.tile([H, 512], dt.float32, tag="hf", bufs=2)
                        nc.vector.tensor_tensor(hf[:], bm[:], nn[:], OP.add)
                        stgf = spool.tile([128, 4, H], dt.float32, tag="stgf")
                        for j in range(4):
                            ptf = ptr.tile([128, 128], dt.float32, tag="ptf")
                            nc.tensor.matmul(ptf[:], hf[:, j * 128:(j + 1) * 128],
                                             idnf[:], is_transpose=True)
                            nc.scalar.copy(stgf[:, j, :], ptf[:])
                        nc.sync.dma_start(
                            out_d.rearrange("(t g p) f -> t p g f", p=128, g=4)[T],
                            stgf[:])

                    # AG chunk k is emitted 2 tiles after its data is ready so
                    # the POOL sequencer's wait on the bounce writes is already
                    # satisfied and gather dispatch doesn't stall.
                    if not last:
                        if TPC - 1 <= T - 2 and (T - 2) % TPC == TPC - 1:
                            k = (T - 2) // TPC
                        elif T == NT - 1:
                            k = NAG - 1
                        else:
                            k = None
                        if k is not None:
                            nc.gpsimd.collective_compute(
                                "AllGather", OP.bypass, replica_groups=RG,
                                ins=[bounce[k * CROWS:(k + 1) * CROWS, :]],
                                outs=[ag_dst[k * NCORES * CROWS:(k + 1) * NCORES * CROWS, :]])
    nc.finalize()
    return nc


_CACHE = {}
_last_in_maps = None


def kernel(**inputs):
    x = np.asarray(inputs["x"], np.float32)
    edge_index = np.asarray(inputs["edge_index"])
    W_msg = np.asarray(inputs["W_msg"], np.float32)
    b_msg = np.asarray(inputs["b_msg"], np.float32)
    W_ih = np.asarray(inputs["W_ih"], np.float32)
    W_hh = np.asarray(inputs["W_hh"], np.float32)
    b_ih = np.asarray(inputs["b_ih"], np.float32)
    b_hh = np.asarray(inputs["b_hh"], np.float32)

    pp = _preprocess(edge_index)
    key = (pp["TOTC"], tuple(c for c, _ in pp["callinfo"]))
    if key not in _CACHE:
        _CACHE[key] = _build(pp)
    nc = _CACHE[key]

    xp = np.zeros((NPAD, H), np.float32)
    xp[:N] = x
    # xfull in chunk-major h_full layout
    perm = _rowof(np.arange(NPAD))
    xfull = np.empty((NPAD, H), np.float32)
    xfull[perm] = xp
    xfull = xfull.astype(BF16)
    W_c = W_ih @ W_msg
    v3 = (W_ih @ b_msg).reshape(1, 3 * H)
    bias = np.stack([
        b_ih[0:128] + b_hh[0:128],
        b_ih[128:256] + b_hh[128:256],
        b_ih[256:384],
        b_hh[256:384],
    ], axis=1).astype(np.float32)
    iota = np.broadcast_to(np.arange(H, dtype=np.float32), (H, H)).astype(BF16)
    idn = np.eye(H, dtype=np.float32)

    in_maps = []
    for c in range(NCORES):
        sh = xp[c * SHARD:(c + 1) * SHARD]
        in_maps.append({
            "x_T": np.ascontiguousarray(sh.T).astype(BF16),
            "xfull": xfull,
            "idx": pp["idx16"][c],
            "offs": pp["offs"][c],
            "deg": pp["deg"][c].reshape(1, SHARD).astype(BF16),
            "wct": np.ascontiguousarray(W_c.T).astype(BF16),
            "whht": np.ascontiguousarray(W_hh.T).astype(BF16),
            "v3": v3.astype(BF16),
            "bias": bias,
            "iota": np.ascontiguousarray(iota),
            "idn": idn.astype(BF16),
            "idnf": idn,
        })

    global _last_in_maps
    _last_in_maps = in_maps
    from concourse.bass_utils import run_bass_kernel_spmd
    res = run_bass_kernel_spmd(nc, in_maps, core_ids=list(range(NCORES)))
    outs = res.results
    full = np.concatenate([outs[c]["out"] for c in range(NCORES)], axis=0)
    return full[:N].astype(np.float32)
